# revision 16
# baseline (speedup 1.0000x reference)
"""Trainium2 kernel for nn_Decoder (attention-LSTM decoder, B=32 LX=128 TY=64 D=512 V=32000).

Math used (exact reformulations of the reference):
  - scores[b,l] = x_enc[b,l,:]@v + alpha[b] with v = w_att @ enc_to_k_w; the
    alpha term is constant over l, and softmax is shift-invariant per row, so
    the attention weights (and hence ctx, r_c) are constant over all 64 steps.
  - gates split: y-embedding part precomputed for all steps (host GEMM);
    per-step device work = [input_feed, h] @ W_fh.T (+ inject of the
    precomputed part into PSUM), LSTM pointwise, pre_readout tanh.
  - readout logits computed on-device as one big matmul, vocab-sharded
    across the 8 cores (all-gather-free local logits).
Per-core device program is identical (SPMD); only the readout shard and
output slice differ.
"""

import numpy as np
import ml_dtypes

import concourse.bass as bass
import concourse.bacc as bacc
import concourse.mybir as mybir
import concourse.tile as tile
from concourse.bass_utils import run_bass_kernel_spmd

BF16 = mybir.dt.bfloat16
F32 = mybir.dt.float32
AF = mybir.ActivationFunctionType

B, LX, TY, D, V = 32, 128, 64, 512, 32000
NC = 8
VS = V // NC  # 4000 vocab rows per core
NEG_INF = 1e9

_CACHE = {}


def _build_bass():
    nc = bacc.Bacc("TRN2", target_bir_lowering=False, debug=False, num_devices=NC)

    # DRAM I/O (per-core SPMD; same names on every core)
    gy_d = nc.dram_tensor("gy", [TY // 4, 128, 4 * D], BF16, kind="ExternalInput")
    wp_d = nc.dram_tensor("wp", [8, 128, 4 * D], BF16, kind="ExternalInput")
    wrh_d = nc.dram_tensor("wrh", [4, 128, D], BF16, kind="ExternalInput")
    rt_d = nc.dram_tensor("rt", [4, 128, VS], BF16, kind="ExternalInput")
    rc_d = nc.dram_tensor("rc", [B, D], BF16, kind="ExternalInput")
    h0t_d = nc.dram_tensor("h0t", [4, 128, B], BF16, kind="ExternalInput")
    c0_d = nc.dram_tensor("c0", [B, D], F32, kind="ExternalInput")
    iden_d = nc.dram_tensor("iden", [128, B], BF16, kind="ExternalInput")
    out_d = nc.dram_tensor("out", [TY * B, VS], F32, kind="ExternalOutput")

    with tile.TileContext(nc) as tc:
        with (
            tc.tile_pool(name="const", bufs=1) as cpool,
            tc.tile_pool(name="state", bufs=1) as spool,
            tc.tile_pool(name="work", bufs=2) as work,
            tc.tile_pool(name="sbuf2", bufs=2) as sbuf2,
            tc.tile_pool(name="ops", bufs=3) as ops,
            tc.tile_pool(name="ps_g", bufs=1, space="PSUM") as ps_g,
            tc.tile_pool(name="ps_t", bufs=2, space="PSUM") as ps_t,
            tc.tile_pool(name="ps_a", bufs=1, space="PSUM") as ps_a,
            tc.tile_pool(name="ps_r", bufs=1, space="PSUM") as ps_r,
        ):
            # ---- resident constants ----
            wp = []
            for k in range(8):
                t = cpool.tile([128, 4 * D], BF16, tag=f"wp{k}")
                nc.sync.dma_start(t[:], wp_d[k])
                wp.append(t)
            wrh = []
            for k in range(4):
                t = cpool.tile([128, D], BF16, tag=f"wrh{k}")
                nc.sync.dma_start(t[:], wrh_d[k])
                wrh.append(t)
            rc = cpool.tile([B, D], BF16, tag="rc")
            nc.sync.dma_start(rc[:], rc_d[:])
            iden = cpool.tile([128, B], BF16, tag="iden")
            nc.sync.dma_start(iden[:], iden_d[:])

            # ---- state (ping-pong buffered to keep per-inst wait lists small) ----
            hT = sbuf2.tile([128, 128], BF16, tag="hT")  # col block k = h chunk k, [128,32]
            nc.sync.dma_start(hT[:, 0:32], h0t_d[0])
            nc.sync.dma_start(hT[:, 32:64], h0t_d[1])
            nc.sync.dma_start(hT[:, 64:96], h0t_d[2])
            nc.sync.dma_start(hT[:, 96:128], h0t_d[3])
            c_st = cpool.tile([B, D], F32, tag="c_st")
            nc.sync.dma_start(c_st[:], c0_d[:])
            c_sb = sbuf2.tile([B, D], F32, tag="c")
            nc.scalar.copy(c_sb[:], c_st[:])
            # pre-readout history, transposed, chunk-major:
            # prT[:, 2048*k + 32*t : +32] = (pre_readout_t chunk k).T
            prT = spool.tile([128, 4 * TY * B], BF16, tag="prT")

            gyt = []
            for m in range(TY // 4):
                t = cpool.tile([128, 4 * D], BF16, tag=f"gyt{m}")
                nc.sync.dma_start(t[:], gy_d[m])
                gyt.append(t)
            rt = []
            for k in range(4):
                t = cpool.tile([128, VS], BF16, tag=f"rt{k}")
                nc.sync.dma_start(t[:], rt_d[k])
                rt.append(t)

            def readout_mtile(m):
                # rows 128m..128m+128 of out = steps 4m..4m+3
                for n in range(8):
                    rps = ps_r.tile([128, 500], F32, tag="rps")
                    for k in range(4):
                        nc.tensor.matmul(
                            rps[:],
                            prT[:, 2048 * k + 128 * m : 2048 * k + 128 * m + 128],
                            rt[k][:, 500 * n : 500 * n + 500],
                            start=(k == 0),
                            stop=(k == 3),
                        )
                    o_sb = ops.tile([128, 500], F32, tag="osb")
                    nc.scalar.activation(o_sb[:], rps[:], AF.Copy)
                    nc.sync.dma_start(
                        out_d[128 * m : 128 * m + 128, 500 * n : 500 * n + 500], o_sb[:]
                    )

            for t in range(TY):
                gy = gyt[t // 4][32 * (t % 4) : 32 * (t % 4) + 32, :]

                gps = ps_g.tile([B, 4 * D], F32, tag="gps")
                for n in range(4):
                    nsl = slice(512 * n, 512 * n + 512)
                    # inject precomputed y-embedding gate part: I.T @ gy = gy
                    nc.tensor.matmul(
                        gps[:, nsl], iden[32 * (t % 4) : 32 * (t % 4) + 32, :], gy[:, nsl],
                        start=True, stop=False, tile_position=(32 * (t % 4), 0),
                    )
                    for k in range(8):
                        if t == 0 and k < 4:
                            continue  # input_feed_0 == 0
                        if k < 4:
                            lhsT = prT[:, 2048 * k + 32 * (t - 1) : 2048 * k + 32 * (t - 1) + 32]
                        else:
                            lhsT = hT[:, 32 * (k - 4) : 32 * (k - 4) + 32]
                        nc.tensor.matmul(
                            gps[:, nsl], lhsT, wp[k][:, nsl], start=False, stop=(k == 7)
                        )

                # gate order is [i, f, o, g] (host permuted)
                s_ifo = work.tile([B, 3 * D], F32, tag="sifo")
                nc.scalar.activation(s_ifo[:], gps[:, 0 : 3 * D], AF.Sigmoid)
                s_g = work.tile([B, D], F32, tag="sg")
                nc.scalar.activation(s_g[:], gps[:, 3 * D : 4 * D], AF.Tanh)

                t1 = work.tile([B, D], F32, tag="t1")
                nc.vector.tensor_mul(t1[:], s_ifo[:, 0:D], s_g[:])  # i*g
                t2 = work.tile([B, D], F32, tag="t2")
                nc.vector.tensor_mul(t2[:], s_ifo[:, D : 2 * D], c_sb[:])  # f*c
                c_new = sbuf2.tile([B, D], F32, tag="c")
                nc.vector.tensor_add(c_new[:], t2[:], t1[:])
                c_sb = c_new
                tcell = work.tile([B, D], F32, tag="tc")
                nc.scalar.activation(tcell[:], c_sb[:], AF.Tanh)
                h_b = work.tile([B, D], BF16, tag="hb")
                nc.vector.tensor_mul(h_b[:], s_ifo[:, 2 * D : 3 * D], tcell[:])  # o*tanh(c)

                # transpose h -> hT  (PE identity transpose, 4 chunks)
                tps = ps_t.tile([128, 128], F32, tag="tps")
                for k in range(4):
                    nc.tensor.matmul(
                        tps[:, 32 * k : 32 * k + 32],
                        h_b[:, 128 * k : 128 * k + 128],
                        iden[0:32, :],
                        start=True,
                        stop=True,
                    )
                hT = sbuf2.tile([128, 128], BF16, tag="hT")
                nc.scalar.activation(hT[:], tps[:], AF.Copy)

                # pre_readout = tanh(h @ Wrh.T + rc); rc injected via identity
                aps = ps_a.tile([B, D], F32, tag="aps")
                nc.tensor.matmul(aps[:], iden[0:32, :], rc[:], start=True, stop=False)
                for k in range(4):
                    nc.tensor.matmul(
                        aps[:],
                        hT[:, 32 * k : 32 * k + 32],
                        wrh[k][:],
                        start=False,
                        stop=(k == 3),
                    )
                if_b = work.tile([B, D], BF16, tag="ifb")
                nc.scalar.activation(if_b[:], aps[:], AF.Tanh)

                # transpose pre_readout -> prT columns for step t
                ips = ps_t.tile([128, 128], F32, tag="tps")
                for k in range(4):
                    nc.tensor.matmul(
                        ips[:, 32 * k : 32 * k + 32],
                        if_b[:, 128 * k : 128 * k + 128],
                        iden[0:32, :],
                        start=True,
                        stop=True,
                    )
                for k in range(4):
                    nc.scalar.activation(
                        prT[:, 2048 * k + 32 * t : 2048 * k + 32 * t + 32],
                        ips[:, 32 * k : 32 * k + 32],
                        AF.Copy,
                    )

                if t % 4 == 3:
                    readout_mtile(t // 4)

    nc.finalize()
    return nc


def _prep_inputs(x_enc, dec_h0, dec_c0, x_mask, y_train, y_mask,
                 enc_to_k_w, w_trg_w, w_trg_b, w_att_w, w_att_b,
                 ctx_to_readout_w, readout_w, word_emb,
                 lstm_w_ih, lstm_w_hh, lstm_b_ih, lstm_b_hh):
    f32 = np.float32
    bf16 = ml_dtypes.bfloat16
    x_enc = np.asarray(x_enc, f32)

    # attention is constant across steps (softmax shift-invariance)
    v_att = np.asarray(w_att_w, f32)[0] @ np.asarray(enc_to_k_w, f32)  # [2D]
    s_pre = x_enc @ v_att  # [B, LX]
    s_pre = np.where(np.asarray(x_mask, bool), f32(-NEG_INF), s_pre)
    e = np.exp(s_pre - s_pre.max(axis=-1, keepdims=True))
    att = e / e.sum(axis=-1, keepdims=True)
    ctx = np.einsum("bl,bld->bd", att, x_enc).astype(f32)  # [B, 2D]
    c2r = np.asarray(ctx_to_readout_w, f32)
    rc = ctx @ c2r[:, D:].T  # [B, D]
    wrh = c2r[:, :D]  # [D, D]

    # y-embedding gate contribution, all steps at once (+ both biases)
    w_ih = np.asarray(lstm_w_ih, f32)
    w_hh = np.asarray(lstm_w_hh, f32)
    beta = np.asarray(lstm_b_ih, f32) + np.asarray(lstm_b_hh, f32)
    emb = np.asarray(word_emb, f32)[np.asarray(y_train)]  # [B, TY, D]
    gy = emb @ w_ih[:, :D].T + beta  # [B, TY, 4D]

    # gate permutation [i,f,g,o] -> [i,f,o,g]
    perm = np.concatenate(
        [np.arange(0, D), np.arange(D, 2 * D), np.arange(3 * D, 4 * D), np.arange(2 * D, 3 * D)]
    )
    gy = np.ascontiguousarray(np.swapaxes(gy[:, :, perm], 0, 1))  # [TY, B, 4D]
    w_f = w_ih[:, D:]  # input_feed part [4D, D]
    wp = np.concatenate([w_f[perm].T, w_hh[perm].T], axis=0)  # [1024, 4D]

    base = {
        "gy": gy.reshape(TY // 4, 128, 4 * D).astype(bf16),
        "wp": wp.reshape(8, 128, 4 * D).astype(bf16),
        "wrh": np.ascontiguousarray(wrh.T).reshape(4, 128, D).astype(bf16),
        "rc": rc.astype(bf16),
        "h0t": np.ascontiguousarray(np.asarray(dec_h0, f32).T).reshape(4, 128, B).astype(bf16),
        "c0": np.asarray(dec_c0, f32),
        "iden": np.tile(np.eye(B, dtype=f32), (4, 1)).astype(bf16),
    }
    rw = np.asarray(readout_w, f32)
    in_maps = []
    for j in range(NC):
        m = dict(base)
        m["rt"] = np.ascontiguousarray(rw[j * VS : (j + 1) * VS].T).reshape(4, 128, VS).astype(bf16)
        in_maps.append(m)
    return in_maps


def kernel(**inputs) -> np.ndarray:
    if "nc" not in _CACHE:
        _CACHE["nc"] = _build_bass()
    nc = _CACHE["nc"]
    in_maps = _prep_inputs(**inputs)
    res = run_bass_kernel_spmd(nc, in_maps, core_ids=list(range(NC)))
    _CACHE["last_results"] = res
    parts = [res.results[j]["out"].reshape(TY, B, VS) for j in range(NC)]
    logits = np.concatenate(parts, axis=2)  # [TY, B, V]
    return np.ascontiguousarray(np.swapaxes(logits, 0, 1)).astype(np.float32)
